# revision 16
# baseline (speedup 1.0000x reference)
"""DMFNet DAEM module kernel for Trainium2 (8 NeuronCores, batch-parallel).

Sharding: data-parallel over batch B=8 -> 1 image per core, weights replicated,
no collectives.

Per-core layout conventions:
  - "stacked" channel-major [128, 8192]: rows 0:64 = channels of positions
    n < 8192 (h < 64), rows 64:128 = positions n >= 8192; n = h*128 + w.
  - q/k "packed" [128, 1024] bf16: rows 8g:8g+8 hold 8 q/k channels of
    positions g*1024 .. (g+1)*1024 (A-order: n = h*128+w; B-order: w*128+h).
    Slice h lives at [8*(h//8):8*(h//8)+8, (h%8)*128:(h%8+1)*128].
  - vT spatial-major bf16: A-dir: 64 pair-tiles [128, 2, 65] (col 64 = ones,
    tile t half q covers slice h = t + 64*q); B-dir: 128 tiles [128, 65].
"""

import numpy as np

B, C, H, W = 8, 64, 128, 128
N = H * W            # 16384
NH = N // 2          # 8192
NEG_B = 30.0         # diagonal mask magnitude: exp(-30) ~ 0 in bf16

_CACHE = {}


def _f32(a):
    return np.ascontiguousarray(np.asarray(a, dtype=np.float32))


def _prepare_weights(params):
    """Fold affines into conv weights host-side (deferred-scale trick)."""
    p = {}
    for k, v in params.items():
        if isinstance(v, dict):
            p[k] = {kk: _f32(vv) for kk, vv in v.items()}
        else:
            p[k] = _f32(v)
    sa = p['sa']
    gamma = float(np.asarray(params['sa']['gamma']))

    out = {}

    def bd(wT):
        z = np.zeros((128, 128), np.float32)
        z[:64, :64] = wT
        z[64:, 64:] = wT
        return z

    def stack2(v):
        return np.concatenate([v, v]).reshape(128, 1).astype(np.float32)

    chains, extras, sigma_y = [], [], []

    def basic_fold(cp, pre_scale, pre_bias):
        Wc, s, b = cp['w'], cp['s'], cp['b']
        W_eff = Wc * pre_scale[None, :]
        psum_bias = Wc @ pre_bias
        assert np.all(s > 0), "affine scale must be positive"
        return W_eff, psum_bias + b / s, s

    ident = np.ones(64, np.float32)
    zero = np.zeros(64, np.float32)

    # branch 0
    W1, b1, s1 = basic_fold(p['b0c0'], ident, zero)
    bn = p['b0bn']
    assert np.all(bn['s'] > 0)
    chains.append([(W1, b1)])
    extras.append([bn['b'] / (bn['s'] * s1)])
    sigma_y.append(bn['s'] * s1)

    # branch 1
    W1, b1, s1 = basic_fold(p['b1c0'], ident, zero)
    bn = p['b1bn']
    W2, b2, s2 = basic_fold(p['b1c1'], bn['s'] * s1, bn['b'])
    chains.append([(W1, b1), (W2, b2)])
    extras.append([])
    sigma_y.append(s2)

    # branch 2
    W1, b1, s1 = basic_fold(p['b2c0'], ident, zero)
    bn0, bn1 = p['b2bn0'], p['b2bn1']
    W2, b2, s2 = basic_fold(p['b2c1'], bn0['s'] * s1, bn0['b'])
    assert np.all(bn1['s'] > 0)
    chains.append([(W1, b1), (W2, b2)])
    extras.append([bn1['b'] / (bn1['s'] * s2)])
    sigma_y.append(bn1['s'] * s2)

    # branch 3
    W1, b1, s1 = basic_fold(p['b3c0'], ident, zero)
    bn0, bn1 = p['b3bn0'], p['b3bn1']
    W2, b2, s2 = basic_fold(p['b3c1'], bn0['s'] * s1, bn0['b'])
    W3, b3, s3 = basic_fold(p['b3c2'], bn1['s'] * s2, bn1['b'])
    chains.append([(W1, b1), (W2, b2), (W3, b3)])
    extras.append([])
    sigma_y.append(s3)

    for i, ch in enumerate(chains):
        for j, (Wc, bc) in enumerate(ch):
            out[f'wt{i}_{j}'] = bd(Wc.T)
            out[f'bt{i}_{j}'] = stack2(bc)
        for j, eb in enumerate(extras[i]):
            out[f'be{i}_{j}'] = stack2(eb)

    rs = p['res']
    out['wres'] = bd(rs['w'].T)
    out['bres_s'] = stack2(rs['s'])
    out['bres_b'] = stack2(rs['b'])

    for nm in ('bq', 'bk', 'bv'):
        assert np.allclose(np.asarray(sa[nm]), 0.0), f"{nm} must be zero"
    for i in range(4):
        wq = sa['wq'] * sigma_y[i][None, :]
        wk = sa['wk'] * sigma_y[i][None, :]
        wv = sa['wv'] * sigma_y[i][None, :]
        z = np.zeros((128, 32), np.float32)
        z[:64, 0:8] = wq.T
        z[:64, 8:16] = wk.T
        z[64:, 16:24] = wq.T
        z[64:, 24:32] = wk.T
        out[f'wqk{i}'] = z
        zv = np.zeros((128, 128), np.float32)
        zv[:64, :64] = wv.T
        zv[64:, 64:] = wv.T
        out[f'wvA{i}'] = zv
        zb1 = np.zeros((128, 64), np.float32)
        zb1[:64] = wv.T
        zb2 = np.zeros((128, 64), np.float32)
        zb2[64:] = wv.T
        out[f'wvB1_{i}'] = zb1
        out[f'wvB2_{i}'] = zb2

    ct = p['cat']
    assert np.all(ct['s'] > 0)
    for i in range(4):
        out[f'wcat{i}'] = bd(ct['w'][:, 64 * i:64 * (i + 1)].T)
    out['bcat'] = stack2(ct['b'] / ct['s'])
    out['scat'] = stack2(ct['s'])

    eye = np.eye(128, dtype=np.float32)
    out['negbi'] = -NEG_B * eye
    out['eye4'] = np.concatenate([eye] * 4, axis=1)
    out['ones1x64'] = np.ones((1, 64), np.float32)
    for i in range(4):
        out[f'sigy{i}'] = stack2(sigma_y[i])

    return out, gamma


BF16_W = ('negbi', 'eye4')


def _is_bf16_weight(nm):
    return nm in BF16_W or nm.startswith(('wqk', 'wvA', 'wvB', 'wcat'))


def _build_program(gamma):
    import sys
    if '/opt/trn_rl_repo' not in sys.path:
        sys.path.insert(0, '/opt/trn_rl_repo')
    import concourse.bass as bass
    import concourse.bacc as bacc
    import concourse.mybir as mybir
    from concourse.tile import TileContext
    from concourse.mybir import AluOpType as alu

    dt = mybir.dt
    f32, f32r, bf16 = dt.float32, dt.float32r, dt.bfloat16
    AF = mybir.ActivationFunctionType

    nc = bacc.Bacc(None)

    x_dram = nc.dram_tensor("x_im", [64, N], f32r, kind="ExternalInput")
    out_dram = nc.dram_tensor("out", [64, N], f32, kind="ExternalOutput")

    conv_counts = [1, 2, 2, 3]
    extra_counts = [1, 0, 1, 0]
    wshapes = {}
    for i in range(4):
        for j in range(conv_counts[i]):
            wshapes[f'wt{i}_{j}'] = [128, 128]
            wshapes[f'bt{i}_{j}'] = [128, 1]
        for j in range(extra_counts[i]):
            wshapes[f'be{i}_{j}'] = [128, 1]
        wshapes[f'wqk{i}'] = [128, 32]
        wshapes[f'wvA{i}'] = [128, 128]
        wshapes[f'wvB1_{i}'] = [128, 64]
        wshapes[f'wvB2_{i}'] = [128, 64]
        wshapes[f'wcat{i}'] = [128, 128]
        wshapes[f'sigy{i}'] = [128, 1]
    wshapes.update({'wres': [128, 128], 'bres_s': [128, 1], 'bres_b': [128, 1],
                    'bcat': [128, 1], 'scat': [128, 1], 'ones1x64': [1, 64],
                    'negbi': [128, 128], 'eye4': [128, 512]})

    def _wdt(nm):
        if _is_bf16_weight(nm):
            return bf16
        if nm.startswith('wt') or nm in ('wres', 'ones1x64'):
            return f32r
        return f32
    dram_w = {nm: nc.dram_tensor(nm, shp, _wdt(nm), kind="ExternalInput")
              for nm, shp in wshapes.items()}

    # internal DRAM scratch
    d_scratch = nc.dram_tensor("d_scratch", [N], f32)
    r_scratch = nc.dram_tensor("r_scratch", [N], f32r)
    cat_bounce = nc.dram_tensor("cat_bounce", [128, NH], f32)

    NCHUNK = 512
    nchunks = NH // NCHUNK   # 16

    from contextlib import ExitStack
    with TileContext(nc) as tc, ExitStack() as ctx:
        wpool = ctx.enter_context(tc.tile_pool(name="weights", bufs=1))
        big = ctx.enter_context(tc.tile_pool(name="big", bufs=1))
        psum = ctx.enter_context(tc.tile_pool(name="psum", bufs=2,
                                              space="PSUM"))
        sm = ctx.enter_context(tc.tile_pool(name="small", bufs=2))
        ring = ctx.enter_context(tc.tile_pool(name="ring", bufs=3))

        wsb = {}
        for nm, shp in wshapes.items():
            t = wpool.tile(shp, _wdt(nm), name=f"w_{nm}", tag=f"w_{nm}")
            nc.sync.dma_start(out=t, in_=dram_w[nm][:])
            wsb[nm] = t

        # persistent buffers (per-partition free bytes in comments)
        t_st = big.tile([128, NH], f32r, name="t_st", tag="t_st")       # 32K
        y16 = big.tile([128, NH], bf16, name="y16", tag="y16")         # 16K
        # q/k packs: rows 32a:32a+8 hold channels of positions a*4096..,
        # slice h at [32*(h//32) : +8, (h%32)*128 : +128]
        qpA = big.tile([128, 4096], bf16, name="qpA", tag="qpA")       # 8K
        kpA = big.tile([128, 4096], bf16, name="kpA", tag="kpA")       # 8K
        qpB = big.tile([128, 4096], bf16, name="qpB", tag="qpB")       # 8K
        kpB = big.tile([128, 4096], bf16, name="kpB", tag="kpB")       # 8K
        vT = big.tile([128, 8320], bf16, name="vT", tag="vT")          # 16.6K
        O_acc = big.tile([65, N], bf16, name="O_acc", tag="O_acc")     # 32K
        dcam0 = big.tile([128, NH], bf16, name="dcam0", tag="dcam0")   # 16K
        dcam1 = big.tile([128, NH], bf16, name="dcam1", tag="dcam1")   # 16K
        dmat = big.tile([128, 128], f32, name="dmat", tag="dmat")      # 0.5K
        rmat = big.tile([128, 128], f32, name="rmat", tag="rmat")      # 0.5K
        # total ~152K + weights ~7K + rings/small pools

        def conv_from(dst, src_fn, wname, bias_ap, extra_bias=None):
            """dst[:, chunk] = max(W.T @ src + bias, 0) per 512-chunk."""
            wap = wsb[wname]
            for n in range(nchunks):
                sl = slice(n * NCHUNK, (n + 1) * NCHUNK)
                rhs = src_fn(n, sl)
                ps = psum.tile([128, NCHUNK], f32, name="mm_ps", tag="mm_ps")
                nc.tensor.matmul(ps, wap, rhs, start=True, stop=True)
                nc.vector.tensor_scalar(dst[:, sl], ps, bias_ap, 0.0,
                                        alu.add, alu.max)

        def x_rhs(n, sl):
            xr = ring.tile([128, NCHUNK], f32r, name="xr", tag="r512")
            nc.sync.dma_start(out=xr[0:64, :], in_=x_dram[:, sl])
            nc.sync.dma_start(out=xr[64:128, :],
                              in_=x_dram[:, NH + sl.start:NH + sl.stop])
            return xr

        def t_rhs(n, sl):
            return t_st[:, sl]

        for br in range(4):
            # ---------------- trunk chain ----------------
            for j in range(conv_counts[br]):
                conv_from(t_st, x_rhs if j == 0 else t_rhs, f'wt{br}_{j}',
                          wsb[f'bt{br}_{j}'])
            for j in range(extra_counts[br]):
                nc.vector.tensor_scalar(t_st[:], t_st[:], wsb[f'be{br}_{j}'],
                                        0.0, alu.add, alu.max)
            nc.vector.tensor_scalar(y16[:], t_st[:], 0.0, None, alu.add)

            # ---------------- qk convs + unspread ----------------
            qk_st = big.tile([32, NH], bf16, name="qk_st", tag="qkor")
            wqk = wsb[f'wqk{br}']
            # A layout
            for n in range(nchunks):
                sl = slice(n * NCHUNK, (n + 1) * NCHUNK)
                ps = psum.tile([32, NCHUNK], f32, name="mm_ps", tag="mm_ps")
                nc.tensor.matmul(ps, wqk, y16[:, sl], start=True, stop=True)
                nc.vector.tensor_scalar(qk_st[:, sl], ps, 0.0, None, alu.add)
            for a in range(4):
                r0, c0 = (0, a * 4096) if a < 2 else (16, (a - 2) * 4096)
                sl = slice(c0, c0 + 4096)
                nc.sync.dma_start(out=qpA[32 * a:32 * a + 8, :],
                                  in_=qk_st[r0:r0 + 8, sl])
                nc.sync.dma_start(out=kpA[32 * a:32 * a + 8, :],
                                  in_=qk_st[r0 + 8:r0 + 16, sl])
            # B layout (transposed column order via 2D AP)
            yb = y16.rearrange("p (hh w) -> p w hh", hh=64, w=128)
            for n in range(nchunks):
                sl = slice(n * NCHUNK, (n + 1) * NCHUNK)
                ps = psum.tile([32, NCHUNK], f32, name="mm_ps", tag="mm_ps")
                nc.tensor.matmul(ps, wqk, yb[:, n * 8:(n + 1) * 8, :],
                                 start=True, stop=True)
                nc.vector.tensor_scalar(qk_st[:, sl], ps, 0.0, None, alu.add)
            # qk_st cols are (w, h') pairs, h' in 0:64; spread to B-order
            # packed: group a covers w in [a*32, (a+1)*32), cols w%32*128 + h
            qk3 = qk_st.rearrange("p (w hh) -> p w hh", w=128, hh=64)
            dstq = qpB.rearrange("p (w h) -> p w h", w=32, h=128)
            dstk = kpB.rearrange("p (w h) -> p w h", w=32, h=128)
            for a in range(4):
                wlo = a * 32
                for half, rq in ((0, 0), (1, 16)):
                    nc.sync.dma_start(
                        out=dstq[32 * a:32 * a + 8, :,
                                 64 * half:64 * half + 64],
                        in_=qk3[rq:rq + 8, wlo:wlo + 32, :])
                    nc.sync.dma_start(
                        out=dstk[32 * a:32 * a + 8, :,
                                 64 * half:64 * half + 64],
                        in_=qk3[rq + 8:rq + 16, wlo:wlo + 32, :])

            # ---------------- vT build, A-direction ----------------
            wvA = wsb[f'wvA{br}']
            vT_A = vT.rearrange("p (t two c) -> p t two c", t=64, two=2, c=65)
            for t in range(64):
                ps = psum.tile([128, 128], f32, name="vt_ps", tag="vt_ps")
                nc.tensor.matmul(ps, y16[:, t * 128:(t + 1) * 128], wvA,
                                 start=True, stop=True)
                nc.vector.tensor_scalar(
                    vT_A[:, t, :, 0:64],
                    ps.rearrange("p (two c) -> p two c", two=2, c=64),
                    0.0, None, alu.add)
            nc.vector.memset(vT_A[:, :, :, 64:65], 1.0)

            # ---------------- attention, W direction ----------------
            vT5 = vT.rearrange("p (t c) -> p t c", t=128, c=65)
            for g in range(32):
                eps = psum.tile([128, 512], f32, name="e_ps", tag="e_ps")
                for s in range(4):
                    h = g * 4 + s
                    base = 32 * (h // 32)
                    co = (h % 32) * 128
                    nc.tensor.matmul(eps[:, s * 128:(s + 1) * 128],
                                     kpA[base:base + 8, co:co + 128],
                                     qpA[base:base + 8, co:co + 128],
                                     start=True, stop=True,
                                     tile_position=(base, 0))
                at = sm.tile([128, 512], bf16, name="at", tag="at", bufs=2)
                nc.scalar.activation(at, eps, AF.Exp)
                ops = psum.tile([65, 512], f32, name="o_ps", tag="o_ps")
                for s in range(4):
                    h = g * 4 + s
                    nc.tensor.matmul(ops[:, s * 128:(s + 1) * 128],
                                     vT5[:, (h % 64) * 2 + (h // 64), :],
                                     at[:, s * 128:(s + 1) * 128],
                                     start=True, stop=True)
                nc.vector.tensor_scalar(O_acc[:, g * 512:(g + 1) * 512],
                                        ops, 0.0, None, alu.add)

            # ---------------- vT build, B-direction (reuses vT) ----------
            yv = y16.rearrange("p (hh w) -> p w hh", hh=64, w=128)
            wvB1, wvB2 = wsb[f'wvB1_{br}'], wsb[f'wvB2_{br}']
            vT_B = vT.rearrange("p (w c) -> p w c", w=128, c=65)
            for w in range(128):
                ps = psum.tile([128, 65], f32, name="vt_ps", tag="vt_ps")
                lhsT = yv[:, w, :]
                nc.tensor.matmul(ps[0:64, 0:64], lhsT, wvB1, start=True,
                                 stop=True)
                nc.tensor.matmul(ps[64:128, 0:64], lhsT, wvB2, start=True,
                                 stop=True, tile_position=(0, 64))
                nc.vector.tensor_scalar(vT_B[:, w, 0:64], ps[:, 0:64],
                                        0.0, None, alu.add)
            nc.vector.memset(vT_B[:, 0:8320 // 65, 64:65], 1.0)

            # ---------------- attention, H direction ----------------
            o3 = O_acc.rearrange("p (h w) -> p h w", h=128, w=128)
            for g in range(32):
                eps = psum.tile([128, 512], f32, name="e_ps", tag="e_ps")
                nc.tensor.matmul(eps[:, 0:512], wsb['negbi'], wsb['eye4'],
                                 start=True, stop=False,
                                 skip_group_check=True)
                for s in range(4):
                    w = g * 4 + s
                    base = 32 * (w // 32)
                    co = (w % 32) * 128
                    nc.tensor.matmul(eps[:, s * 128:(s + 1) * 128],
                                     kpB[base:base + 8, co:co + 128],
                                     qpB[base:base + 8, co:co + 128],
                                     start=False, stop=True,
                                     skip_group_check=True,
                                     tile_position=(base, 0))
                at = sm.tile([128, 512], bf16, name="at", tag="at", bufs=2)
                nc.scalar.activation(at, eps, AF.Exp)
                ops = psum.tile([65, 512], f32, name="o_ps", tag="o_ps")
                for s in range(4):
                    w = g * 4 + s
                    nc.tensor.matmul(ops[:, s * 128:(s + 1) * 128],
                                     vT_B[:, w, :],
                                     at[:, s * 128:(s + 1) * 128],
                                     start=True, stop=True)
                dsto = o3[:, :, g * 4:(g + 1) * 4]
                srco = ops.rearrange("p (s h) -> p h s", s=4, h=128)
                nc.vector.tensor_tensor(dsto, dsto, srco, alu.add)

            # ---------------- denominators -> rmat ----------------
            nc.gpsimd.dma_start(out=d_scratch[:], in_=O_acc[64:65, :])
            nc.sync.dma_start(out=dmat[:],
                              in_=d_scratch.rearrange("(h w) -> h w", h=128))
            nc.vector.reciprocal(rmat[:], dmat[:])
            nc.vector.tensor_scalar(rmat[:], rmat[:], float(gamma), None,
                                    alu.mult)
            nc.gpsimd.dma_start(out=r_scratch[:], in_=rmat[:])

            # ---------------- epilogue ----------------
            OR_st = big.tile([128, NH], bf16, name="OR_st", tag="qkor")
            dcam = dcam0 if br in (0, 2) else dcam1
            for n in range(32):
                sl = slice(n * NCHUNK, (n + 1) * NCHUNK)
                rr = ring.tile([1, NCHUNK], f32r, name="rr", tag="rr", bufs=2)
                nc.sync.dma_start(out=rr,
                                  in_=r_scratch.rearrange("(o n) -> o n",
                                                          o=1)[:, sl])
                rp = psum.tile([64, NCHUNK], f32, name="vt_ps", tag="vt_ps")
                nc.tensor.matmul(rp, wsb['ones1x64'],
                                 rr, start=True, stop=True)
                ortmp = sm.tile([64, NCHUNK], bf16, name="ortmp", tag="ortmp",
                                bufs=2)
                nc.vector.tensor_tensor(ortmp, O_acc[0:64, sl], rp, alu.mult)
                if n < 16:
                    nc.sync.dma_start(out=OR_st[0:64, sl], in_=ortmp)
                else:
                    sl2 = slice((n - 16) * NCHUNK, (n - 15) * NCHUNK)
                    nc.sync.dma_start(out=OR_st[64:128, sl2], in_=ortmp)
            nc.vector.scalar_tensor_tensor(dcam[:], y16[:], wsb[f'sigy{br}'],
                                           OR_st[:], alu.mult, alu.add)

            # ---------------- cat pair 0+1 -> DRAM bounce ----------------
            if br == 1:
                for n in range(nchunks):
                    sl = slice(n * NCHUNK, (n + 1) * NCHUNK)
                    ps = psum.tile([128, NCHUNK], f32, name="mm_ps", tag="mm_ps")
                    nc.tensor.matmul(ps, wsb['wcat0'], dcam0[:, sl],
                                     start=True, stop=False,
                                     skip_group_check=True)
                    nc.tensor.matmul(ps, wsb['wcat1'], dcam1[:, sl],
                                     start=False, stop=True,
                                     skip_group_check=True)
                    cb = ring.tile([128, NCHUNK], f32, name="cb", tag="r512")
                    nc.vector.tensor_scalar(cb, ps, 0.0, None, alu.add)
                    nc.sync.dma_start(out=cat_bounce[:, sl], in_=cb)

        # ---------------- res conv (t_st free after branch 3) ----------
        res_act = t_st
        wap = wsb['wres']
        for n in range(nchunks):
            sl = slice(n * NCHUNK, (n + 1) * NCHUNK)
            ps = psum.tile([128, NCHUNK], f32, name="mm_ps", tag="mm_ps")
            nc.tensor.matmul(ps, wap, x_rhs(n, sl), start=True, stop=True)
            nc.scalar.activation(res_act[:, sl], ps, AF.Relu,
                                 bias=wsb['bres_b'], scale=wsb['bres_s'])

        # ---------------- cat pair 2+3 + final ----------------
        for n in range(nchunks):
            sl = slice(n * NCHUNK, (n + 1) * NCHUNK)
            ps = psum.tile([128, NCHUNK], f32, name="mm_ps", tag="mm_ps")
            nc.tensor.matmul(ps, wsb['wcat2'], dcam0[:, sl],
                             start=True, stop=False, skip_group_check=True)
            nc.tensor.matmul(ps, wsb['wcat3'], dcam1[:, sl],
                             start=False, stop=True, skip_group_check=True)
            cb = ring.tile([128, NCHUNK], f32, name="cb2", tag="r512")
            nc.sync.dma_start(out=cb, in_=cat_bounce[:, sl])
            tmp = sm.tile([128, NCHUNK], f32, name="ftmp", tag="ftmp", bufs=2)
            nc.vector.tensor_tensor(tmp, ps, cb, alu.add)
            nc.vector.tensor_scalar(tmp, tmp, wsb['bcat'], 0.0, alu.add,
                                    alu.max)
            nc.vector.scalar_tensor_tensor(tmp, tmp, wsb['scat'],
                                           res_act[:, sl], alu.mult, alu.add)
            fout = ring.tile([128, NCHUNK], f32, name="fout", tag="r512")
            nc.vector.tensor_scalar(fout, tmp, 0.0, None, alu.max)
            # unstack to DRAM: rows 0:64 -> cols sl; rows 64:128 -> cols+8192
            nc.sync.dma_start(out=out_dram[:, sl], in_=fout[0:64, :])
            nc.sync.dma_start(
                out=out_dram[:, NH + n * NCHUNK:NH + (n + 1) * NCHUNK],
                in_=fout[64:128, :])

    nc.finalize()
    return nc


def kernel(x, params):
    import sys
    if '/opt/trn_rl_repo' not in sys.path:
        sys.path.insert(0, '/opt/trn_rl_repo')
    import ml_dtypes

    x = _f32(x)
    weights, gamma = _prepare_weights(params)

    if 'nc' not in _CACHE:
        _CACHE['nc'] = _build_program(gamma)
    nc = _CACHE['nc']

    from concourse.bass_utils import run_bass_kernel_spmd
    in_maps = []
    for c in range(B):
        m = {'x_im': np.ascontiguousarray(x[c].reshape(64, N))}
        for k, v in weights.items():
            if _is_bf16_weight(k):
                m[k] = np.ascontiguousarray(v.astype(ml_dtypes.bfloat16))
            else:
                m[k] = np.ascontiguousarray(v.astype(np.float32))
        in_maps.append(m)
    res = run_bass_kernel_spmd(nc, in_maps, list(range(B)))
    out = np.stack([res.results[c]['out'] for c in range(B)])
    return out.reshape(B, C, H, W).astype(np.float32)


# revision 23
# speedup vs baseline: 24.2893x; 24.2893x over previous
"""DMFNet DAEM module kernel for Trainium2 (8 NeuronCores, batch-parallel).

Sharding: data-parallel over batch B=8 -> 1 image per core, weights replicated,
no collectives.

Per-core layout conventions:
  - "stacked" channel-major [128, 8192]: rows 0:64 = channels of positions
    n < 8192 (h < 64), rows 64:128 = positions n >= 8192; n = h*128 + w.
  - q/k "packed" [128, 1024] bf16: rows 8g:8g+8 hold 8 q/k channels of
    positions g*1024 .. (g+1)*1024 (A-order: n = h*128+w; B-order: w*128+h).
    Slice h lives at [8*(h//8):8*(h//8)+8, (h%8)*128:(h%8+1)*128].
  - vT spatial-major bf16: A-dir: 64 pair-tiles [128, 2, 65] (col 64 = ones,
    tile t half q covers slice h = t + 64*q); B-dir: 128 tiles [128, 65].
"""

import numpy as np

B, C, H, W = 8, 64, 128, 128
N = H * W            # 16384
NH = N // 2          # 8192
NEG_B = 30.0         # diagonal mask magnitude: exp(-30) ~ 0 in bf16

_CACHE = {}


def _f32(a):
    return np.ascontiguousarray(np.asarray(a, dtype=np.float32))


def _prepare_weights(params):
    """Fold affines into conv weights host-side (deferred-scale trick)."""
    p = {}
    for k, v in params.items():
        if isinstance(v, dict):
            p[k] = {kk: _f32(vv) for kk, vv in v.items()}
        else:
            p[k] = _f32(v)
    sa = p['sa']
    gamma = float(np.asarray(params['sa']['gamma']))

    out = {}

    def bd(wT):
        z = np.zeros((128, 128), np.float32)
        z[:64, :64] = wT
        z[64:, 64:] = wT
        return z

    def stack2(v):
        return np.concatenate([v, v]).reshape(128, 1).astype(np.float32)

    chains, extras, sigma_y = [], [], []

    def basic_fold(cp, pre_scale, pre_bias):
        Wc, s, b = cp['w'], cp['s'], cp['b']
        W_eff = Wc * pre_scale[None, :]
        psum_bias = Wc @ pre_bias
        assert np.all(s > 0), "affine scale must be positive"
        return W_eff, psum_bias + b / s, s

    ident = np.ones(64, np.float32)
    zero = np.zeros(64, np.float32)

    # branch 0
    W1, b1, s1 = basic_fold(p['b0c0'], ident, zero)
    bn = p['b0bn']
    assert np.all(bn['s'] > 0)
    chains.append([(W1, b1)])
    extras.append([bn['b'] / (bn['s'] * s1)])
    sigma_y.append(bn['s'] * s1)

    # branch 1
    W1, b1, s1 = basic_fold(p['b1c0'], ident, zero)
    bn = p['b1bn']
    W2, b2, s2 = basic_fold(p['b1c1'], bn['s'] * s1, bn['b'])
    chains.append([(W1, b1), (W2, b2)])
    extras.append([])
    sigma_y.append(s2)

    # branch 2
    W1, b1, s1 = basic_fold(p['b2c0'], ident, zero)
    bn0, bn1 = p['b2bn0'], p['b2bn1']
    W2, b2, s2 = basic_fold(p['b2c1'], bn0['s'] * s1, bn0['b'])
    assert np.all(bn1['s'] > 0)
    chains.append([(W1, b1), (W2, b2)])
    extras.append([bn1['b'] / (bn1['s'] * s2)])
    sigma_y.append(bn1['s'] * s2)

    # branch 3
    W1, b1, s1 = basic_fold(p['b3c0'], ident, zero)
    bn0, bn1 = p['b3bn0'], p['b3bn1']
    W2, b2, s2 = basic_fold(p['b3c1'], bn0['s'] * s1, bn0['b'])
    W3, b3, s3 = basic_fold(p['b3c2'], bn1['s'] * s2, bn1['b'])
    chains.append([(W1, b1), (W2, b2), (W3, b3)])
    extras.append([])
    sigma_y.append(s3)

    for i, ch in enumerate(chains):
        for j, (Wc, bc) in enumerate(ch):
            out[f'wt{i}_{j}'] = bd(Wc.T)
            out[f'bt{i}_{j}'] = stack2(bc)
        for j, eb in enumerate(extras[i]):
            out[f'be{i}_{j}'] = stack2(eb)

    rs = p['res']
    out['wres'] = bd(rs['w'].T)
    out['bres_s'] = stack2(rs['s'])
    out['bres_b'] = stack2(rs['b'])

    for nm in ('bq', 'bk', 'bv'):
        assert np.allclose(np.asarray(sa[nm]), 0.0), f"{nm} must be zero"
    for i in range(4):
        wq = sa['wq'] * sigma_y[i][None, :]
        wk = sa['wk'] * sigma_y[i][None, :]
        wv = sa['wv'] * sigma_y[i][None, :]
        z = np.zeros((128, 32), np.float32)
        z[:64, 0:8] = wq.T
        z[:64, 8:16] = wk.T
        z[64:, 16:24] = wq.T
        z[64:, 24:32] = wk.T
        out[f'wqk{i}'] = z
        zv = np.zeros((128, 128), np.float32)
        zv[:64, :64] = wv.T
        zv[64:, 64:] = wv.T
        out[f'wvA{i}'] = zv
        zb1 = np.zeros((128, 64), np.float32)
        zb1[:64] = wv.T
        zb2 = np.zeros((128, 64), np.float32)
        zb2[64:] = wv.T
        out[f'wvB1_{i}'] = zb1
        out[f'wvB2_{i}'] = zb2

    ct = p['cat']
    assert np.all(ct['s'] > 0)
    for i in range(4):
        out[f'wcat{i}'] = bd(ct['w'][:, 64 * i:64 * (i + 1)].T)
    out['bcat'] = stack2(ct['b'] / ct['s'])
    out['scat'] = stack2(ct['s'])

    eye = np.eye(128, dtype=np.float32)
    out['negbi'] = -NEG_B * eye
    out['eye4'] = np.concatenate([eye] * 4, axis=1)
    out['ones1x64'] = np.ones((1, 64), np.float32)
    for i in range(4):
        out[f'sigy{i}'] = stack2(sigma_y[i])

    return out, gamma


BF16_W = ('negbi', 'eye4')


def _is_bf16_weight(nm):
    return nm in BF16_W or nm.startswith(('wqk', 'wvA', 'wvB', 'wcat'))


def _build_program(gamma, reps=1):
    import sys
    if '/opt/trn_rl_repo' not in sys.path:
        sys.path.insert(0, '/opt/trn_rl_repo')
    import concourse.bass as bass
    import concourse.bacc as bacc
    import concourse.mybir as mybir
    from concourse.tile import TileContext
    from concourse.mybir import AluOpType as alu

    dt = mybir.dt
    f32, f32r, bf16 = dt.float32, dt.float32r, dt.bfloat16
    AF = mybir.ActivationFunctionType

    nc = bacc.Bacc(None)

    x_dram = nc.dram_tensor("x_im", [64, N], f32r, kind="ExternalInput")
    out_dram = nc.dram_tensor("out", [64, N], f32, kind="ExternalOutput")

    conv_counts = [1, 2, 2, 3]
    extra_counts = [1, 0, 1, 0]
    wshapes = {}
    for i in range(4):
        for j in range(conv_counts[i]):
            wshapes[f'wt{i}_{j}'] = [128, 128]
            wshapes[f'bt{i}_{j}'] = [128, 1]
        for j in range(extra_counts[i]):
            wshapes[f'be{i}_{j}'] = [128, 1]
        wshapes[f'wqk{i}'] = [128, 32]
        wshapes[f'wvA{i}'] = [128, 128]
        wshapes[f'wvB1_{i}'] = [128, 64]
        wshapes[f'wvB2_{i}'] = [128, 64]
        wshapes[f'wcat{i}'] = [128, 128]
        wshapes[f'sigy{i}'] = [128, 1]
    wshapes.update({'wres': [128, 128], 'bres_s': [128, 1], 'bres_b': [128, 1],
                    'bcat': [128, 1], 'scat': [128, 1], 'ones1x64': [1, 64],
                    'negbi': [128, 128], 'eye4': [128, 512]})

    def _wdt(nm):
        if _is_bf16_weight(nm):
            return bf16
        if nm.startswith('wt') or nm in ('wres', 'ones1x64'):
            return f32r
        return f32
    dram_w = {nm: nc.dram_tensor(nm, shp, _wdt(nm), kind="ExternalInput")
              for nm, shp in wshapes.items()}

    # internal DRAM scratch
    d_scratch = nc.dram_tensor("d_scratch", [N], f32)
    r_scratch = nc.dram_tensor("r_scratch", [N], f32r)
    cat_bounce = nc.dram_tensor("cat_bounce", [128, NH], f32)

    NCHUNK = 512
    nchunks = NH // NCHUNK   # 16

    from contextlib import ExitStack
    with TileContext(nc) as tc, ExitStack() as ctx:
        wpool = ctx.enter_context(tc.tile_pool(name="weights", bufs=1))
        big = ctx.enter_context(tc.tile_pool(name="big", bufs=1))
        psum = ctx.enter_context(tc.tile_pool(name="psum", bufs=2,
                                              space="PSUM"))
        sm = ctx.enter_context(tc.tile_pool(name="small", bufs=2))
        ring = ctx.enter_context(tc.tile_pool(name="ring", bufs=3))

        wsb = {}
        for nm, shp in wshapes.items():
            t = wpool.tile(shp, _wdt(nm), name=f"w_{nm}", tag=f"w_{nm}")
            nc.sync.dma_start(out=t, in_=dram_w[nm][:])
            wsb[nm] = t

        # persistent buffers (per-partition free bytes in comments)
        t_st = big.tile([128, NH], f32r, name="t_st", tag="t_st")       # 32K
        y16 = big.tile([128, NH], bf16, name="y16", tag="y16")         # 16K
        # q/k packs: rows 32a:32a+8 hold channels of positions a*4096..,
        # slice h at [32*(h//32) : +8, (h%32)*128 : +128]
        qpA = big.tile([128, 4096], bf16, name="qpA", tag="qpA")       # 8K
        kpA = big.tile([128, 4096], bf16, name="kpA", tag="kpA")       # 8K
        qpB = big.tile([128, 4096], bf16, name="qpB", tag="qpB")       # 8K
        kpB = big.tile([128, 4096], bf16, name="kpB", tag="kpB")       # 8K
        vT = big.tile([128, 8320], bf16, name="vT", tag="vT")          # 16.6K
        O_acc = big.tile([65, N], bf16, name="O_acc", tag="O_acc")     # 32K
        dcam0 = big.tile([128, NH], bf16, name="dcam0", tag="dcam0")   # 16K
        dcam1 = big.tile([128, NH], bf16, name="dcam1", tag="dcam1")   # 16K
        dmat = big.tile([128, 128], f32, name="dmat", tag="dmat")      # 0.5K
        rmat = big.tile([128, 128], f32r, name="rmat", tag="rmat")      # 0.5K
        # total ~152K + weights ~7K + rings/small pools

        def conv_from(dst, src_fn, wname, bias_ap, extra_bias=None):
            """dst[:, chunk] = max(W.T @ src + bias, 0) per 512-chunk."""
            wap = wsb[wname]
            for n in range(nchunks):
                sl = slice(n * NCHUNK, (n + 1) * NCHUNK)
                rhs = src_fn(n, sl)
                ps = psum.tile([128, NCHUNK], f32, name="mm_ps", tag="mm_ps")
                nc.tensor.matmul(ps, wap, rhs, start=True, stop=True)
                nc.vector.tensor_scalar(dst[:, sl], ps, bias_ap, 0.0,
                                        alu.add, alu.max)

        def x_rhs(n, sl):
            xr = ring.tile([128, NCHUNK], f32r, name="xr", tag="r512")
            nc.sync.dma_start(out=xr[0:64, :], in_=x_dram[:, sl])
            nc.sync.dma_start(out=xr[64:128, :],
                              in_=x_dram[:, NH + sl.start:NH + sl.stop])
            return xr

        def t_rhs(n, sl):
            return t_st[:, sl]

        for rep in range(reps):
          for br in range(4):
            # ---------------- trunk chain ----------------
            for j in range(conv_counts[br]):
                conv_from(t_st, x_rhs if j == 0 else t_rhs, f'wt{br}_{j}',
                          wsb[f'bt{br}_{j}'])
            for j in range(extra_counts[br]):
                nc.vector.tensor_scalar(t_st[:], t_st[:], wsb[f'be{br}_{j}'],
                                        0.0, alu.add, alu.max)
            nc.vector.tensor_scalar(y16[:], t_st[:], 0.0, None, alu.add)

            # ---------------- qk convs + unspread ----------------
            qk_st = big.tile([32, NH], bf16, name="qk_st", tag="qkor")
            wqk = wsb[f'wqk{br}']
            # A layout
            for n in range(nchunks):
                sl = slice(n * NCHUNK, (n + 1) * NCHUNK)
                ps = psum.tile([32, NCHUNK], f32, name="mm_ps", tag="mm_ps")
                nc.tensor.matmul(ps, wqk, y16[:, sl], start=True, stop=True)
                nc.vector.tensor_scalar(qk_st[:, sl], ps, 0.0, None, alu.add)
            for a in range(4):
                r0, c0 = (0, a * 4096) if a < 2 else (16, (a - 2) * 4096)
                sl = slice(c0, c0 + 4096)
                nc.sync.dma_start(out=qpA[32 * a:32 * a + 8, :],
                                  in_=qk_st[r0:r0 + 8, sl])
                nc.sync.dma_start(out=kpA[32 * a:32 * a + 8, :],
                                  in_=qk_st[r0 + 8:r0 + 16, sl])
            # B layout (transposed column order via 2D AP)
            yb = y16.rearrange("p (hh w) -> p w hh", hh=64, w=128)
            for n in range(nchunks):
                sl = slice(n * NCHUNK, (n + 1) * NCHUNK)
                ps = psum.tile([32, NCHUNK], f32, name="mm_ps", tag="mm_ps")
                nc.tensor.matmul(ps, wqk, yb[:, n * 8:(n + 1) * 8, :],
                                 start=True, stop=True)
                nc.vector.tensor_scalar(qk_st[:, sl], ps, 0.0, None, alu.add)
            # qk_st cols are (w, h') pairs, h' in 0:64; spread to B-order
            # packed: group a covers w in [a*32, (a+1)*32), cols w%32*128 + h
            qk3 = qk_st.rearrange("p (w hh) -> p w hh", w=128, hh=64)
            dstq = qpB.rearrange("p (w h) -> p w h", w=32, h=128)
            dstk = kpB.rearrange("p (w h) -> p w h", w=32, h=128)
            for a in range(4):
                wlo = a * 32
                for half, rq in ((0, 0), (1, 16)):
                    nc.sync.dma_start(
                        out=dstq[32 * a:32 * a + 8, :,
                                 64 * half:64 * half + 64],
                        in_=qk3[rq:rq + 8, wlo:wlo + 32, :])
                    nc.sync.dma_start(
                        out=dstk[32 * a:32 * a + 8, :,
                                 64 * half:64 * half + 64],
                        in_=qk3[rq + 8:rq + 16, wlo:wlo + 32, :])

            # ---------------- vT build, A-direction ----------------
            wvA = wsb[f'wvA{br}']
            vT_A = vT.rearrange("p (t two c) -> p t two c", t=64, two=2, c=65)
            for t in range(64):
                ps = psum.tile([128, 128], f32, name="vt_ps", tag="vt_ps")
                nc.tensor.matmul(ps, y16[:, t * 128:(t + 1) * 128], wvA,
                                 start=True, stop=True)
                nc.vector.tensor_scalar(
                    vT_A[:, t, :, 0:64],
                    ps.rearrange("p (two c) -> p two c", two=2, c=64),
                    0.0, None, alu.add)
            nc.vector.memset(vT_A[:, :, :, 64:65], 1.0)

            # ---------------- attention, W direction ----------------
            vT5 = vT.rearrange("p (t c) -> p t c", t=128, c=65)
            for g in range(32):
                eps = psum.tile([128, 512], f32, name="e_ps", tag="e_ps")
                for s in range(4):
                    h = g * 4 + s
                    base = 32 * (h // 32)
                    co = (h % 32) * 128
                    nc.tensor.matmul(eps[:, s * 128:(s + 1) * 128],
                                     kpA[base:base + 8, co:co + 128],
                                     qpA[base:base + 8, co:co + 128],
                                     start=True, stop=True,
                                     tile_position=(base, 0))
                at = sm.tile([128, 512], bf16, name="at", tag="at", bufs=2)
                nc.scalar.activation(at, eps, AF.Exp)
                ops = psum.tile([65, 512], f32, name="o_ps", tag="o_ps")
                for s in range(4):
                    h = g * 4 + s
                    nc.tensor.matmul(ops[:, s * 128:(s + 1) * 128],
                                     vT5[:, (h % 64) * 2 + (h // 64), :],
                                     at[:, s * 128:(s + 1) * 128],
                                     start=True, stop=True)
                nc.vector.tensor_scalar(O_acc[:, g * 512:(g + 1) * 512],
                                        ops, 0.0, None, alu.add)

            # ---------------- vT build, B-direction (reuses vT) ----------
            yv = y16.rearrange("p (hh w) -> p w hh", hh=64, w=128)
            wvB1, wvB2 = wsb[f'wvB1_{br}'], wsb[f'wvB2_{br}']
            vT_B = vT.rearrange("p (w c) -> p w c", w=128, c=65)
            for w in range(128):
                ps = psum.tile([128, 65], f32, name="vt_ps", tag="vt_ps")
                lhsT = yv[:, w, :]
                nc.tensor.matmul(ps[0:64, 0:64], lhsT, wvB1, start=True,
                                 stop=True)
                nc.tensor.matmul(ps[64:128, 0:64], lhsT, wvB2, start=True,
                                 stop=True, tile_position=(0, 64))
                nc.vector.tensor_scalar(vT_B[:, w, 0:64], ps[:, 0:64],
                                        0.0, None, alu.add)
            nc.vector.memset(vT_B[:, 0:8320 // 65, 64:65], 1.0)

            # ---------------- attention, H direction ----------------
            o3 = O_acc.rearrange("p (h w) -> p h w", h=128, w=128)
            for g in range(32):
                eps = psum.tile([128, 512], f32, name="e_ps", tag="e_ps")
                nc.tensor.matmul(eps[:, 0:512], wsb['negbi'], wsb['eye4'],
                                 start=True, stop=False,
                                 skip_group_check=True)
                for s in range(4):
                    w = g * 4 + s
                    base = 32 * (w // 32)
                    co = (w % 32) * 128
                    nc.tensor.matmul(eps[:, s * 128:(s + 1) * 128],
                                     kpB[base:base + 8, co:co + 128],
                                     qpB[base:base + 8, co:co + 128],
                                     start=False, stop=True,
                                     skip_group_check=True,
                                     tile_position=(base, 0))
                at = sm.tile([128, 512], bf16, name="at", tag="at", bufs=2)
                nc.scalar.activation(at, eps, AF.Exp)
                ops = psum.tile([65, 512], f32, name="o_ps", tag="o_ps")
                for s in range(4):
                    w = g * 4 + s
                    nc.tensor.matmul(ops[:, s * 128:(s + 1) * 128],
                                     vT_B[:, w, :],
                                     at[:, s * 128:(s + 1) * 128],
                                     start=True, stop=True)
                dsto = o3[:, :, g * 4:(g + 1) * 4]
                srco = ops.rearrange("p (s h) -> p h s", s=4, h=128)
                nc.vector.tensor_tensor(dsto, dsto, srco, alu.add)

            # ---------------- denominators -> rmat ----------------
            nc.gpsimd.dma_start(out=d_scratch[:], in_=O_acc[64:65, :])
            nc.sync.dma_start(out=dmat[:],
                              in_=d_scratch.rearrange("(h w) -> h w", h=128))
            nc.vector.reciprocal(dmat[:], dmat[:])
            nc.vector.tensor_scalar(rmat[:], dmat[:], float(gamma), None,
                                    alu.mult)
            nc.sync.dma_start(out=r_scratch.rearrange("(h w) -> h w", h=128),
                              in_=rmat[:])


            # ---------------- epilogue ----------------
            OR_st = big.tile([128, NH], bf16, name="OR_st", tag="qkor")
            dcam = dcam0 if br in (0, 2) else dcam1
            for n in range(32):
                sl = slice(n * NCHUNK, (n + 1) * NCHUNK)
                rr = ring.tile([1, NCHUNK], f32r, name="rr", tag="rr", bufs=2)
                nc.sync.dma_start(out=rr,
                                  in_=r_scratch.rearrange("(o n) -> o n",
                                                          o=1)[:, sl])
                rp = psum.tile([64, NCHUNK], f32, name="vt_ps", tag="vt_ps")
                nc.tensor.matmul(rp, wsb['ones1x64'],
                                 rr, start=True, stop=True)
                ortmp = sm.tile([64, NCHUNK], bf16, name="ortmp", tag="ortmp",
                                bufs=2)
                nc.vector.tensor_tensor(ortmp, O_acc[0:64, sl], rp, alu.mult)
                if n < 16:
                    nc.sync.dma_start(out=OR_st[0:64, sl], in_=ortmp)
                else:
                    sl2 = slice((n - 16) * NCHUNK, (n - 15) * NCHUNK)
                    nc.sync.dma_start(out=OR_st[64:128, sl2], in_=ortmp)
            nc.vector.scalar_tensor_tensor(dcam[:], y16[:], wsb[f'sigy{br}'],
                                           OR_st[:], alu.mult, alu.add)

            # ---------------- cat pair 0+1 -> DRAM bounce ----------------
            if br == 1:
                for n in range(nchunks):
                    sl = slice(n * NCHUNK, (n + 1) * NCHUNK)
                    ps = psum.tile([128, NCHUNK], f32, name="mm_ps", tag="mm_ps")
                    nc.tensor.matmul(ps, wsb['wcat0'], dcam0[:, sl],
                                     start=True, stop=False,
                                     skip_group_check=True)
                    nc.tensor.matmul(ps, wsb['wcat1'], dcam1[:, sl],
                                     start=False, stop=True,
                                     skip_group_check=True)
                    cb = ring.tile([128, NCHUNK], f32, name="cb", tag="r512")
                    nc.vector.tensor_scalar(cb, ps, 0.0, None, alu.add)
                    nc.sync.dma_start(out=cat_bounce[:, sl], in_=cb)

          # ---------------- res conv (t_st free after branch 3) ----------
          if True:
            res_act = t_st
            wap = wsb['wres']
            for n in range(nchunks):
              sl = slice(n * NCHUNK, (n + 1) * NCHUNK)
              ps = psum.tile([128, NCHUNK], f32, name="mm_ps", tag="mm_ps")
              nc.tensor.matmul(ps, wap, x_rhs(n, sl), start=True, stop=True)
              nc.scalar.activation(res_act[:, sl], ps, AF.Relu,
                                   bias=wsb['bres_b'], scale=wsb['bres_s'])

            # ---------------- cat pair 2+3 + final ----------------
            for n in range(nchunks):
              sl = slice(n * NCHUNK, (n + 1) * NCHUNK)
              ps = psum.tile([128, NCHUNK], f32, name="mm_ps", tag="mm_ps")
              nc.tensor.matmul(ps, wsb['wcat2'], dcam0[:, sl],
                               start=True, stop=False, skip_group_check=True)
              nc.tensor.matmul(ps, wsb['wcat3'], dcam1[:, sl],
                               start=False, stop=True, skip_group_check=True)
              cb = ring.tile([128, NCHUNK], f32, name="cb2", tag="r512")
              nc.sync.dma_start(out=cb, in_=cat_bounce[:, sl])
              tmp = sm.tile([128, NCHUNK], f32, name="ftmp", tag="ftmp", bufs=2)
              nc.vector.tensor_tensor(tmp, ps, cb, alu.add)
              nc.vector.tensor_scalar(tmp, tmp, wsb['bcat'], 0.0, alu.add,
                                      alu.max)
              nc.vector.scalar_tensor_tensor(tmp, tmp, wsb['scat'],
                                             res_act[:, sl], alu.mult, alu.add)
              fout = ring.tile([128, NCHUNK], f32, name="fout", tag="r512")
              nc.vector.tensor_scalar(fout, tmp, 0.0, None, alu.max)
              nc.sync.dma_start(out=out_dram[:, sl], in_=fout[0:64, :])
              nc.sync.dma_start(
                  out=out_dram[:, NH + n * NCHUNK:NH + (n + 1) * NCHUNK],
                  in_=fout[64:128, :])

    nc.finalize()
    return nc


def kernel(x, params):
    import sys
    if '/opt/trn_rl_repo' not in sys.path:
        sys.path.insert(0, '/opt/trn_rl_repo')
    import ml_dtypes

    x = _f32(x)
    weights, gamma = _prepare_weights(params)

    if 'nc' not in _CACHE:
        _CACHE['nc'] = _build_program(gamma)
    nc = _CACHE['nc']

    from concourse.bass_utils import run_bass_kernel_spmd
    in_maps = []
    for c in range(B):
        m = {'x_im': np.ascontiguousarray(x[c].reshape(64, N))}
        for k, v in weights.items():
            if _is_bf16_weight(k):
                m[k] = np.ascontiguousarray(v.astype(ml_dtypes.bfloat16))
            else:
                m[k] = np.ascontiguousarray(v.astype(np.float32))
        in_maps.append(m)
    res = run_bass_kernel_spmd(nc, in_maps, list(range(B)))
    out = np.stack([res.results[c]['out'] for c in range(B)])
    return out.reshape(B, C, H, W).astype(np.float32)
